# revision 13
# baseline (speedup 1.0000x reference)
"""Trainium2 Bass kernel for nn_HRPLoss (topk_masking).

Contract: kernel(**inputs) takes FULL unsharded numpy inputs and returns the
FULL output (scalar f32 loss). Pure data parallel: batch dim sharded over 8
NeuronCores; device computes all the heavy streaming reductions; host does the
tiny per-sample top-k masking + final scalar combine (64x17 values).

Layout: each per-core logical volume (9 of them: out1_1, out1_2{x,y}, out2_1,
out2_2{x,y}, targets1, targets2{x,y}) is a flat [BL*K*HW] = [940032] f32 array
viewed as [2176, 432] rows (row r = image i=r//16, chunk ch=r%16; one image =
16 rows of 432 contiguous elements). Tiles are [128, nb*432] bf16 (cast in the
DMA), row r = col-block cb=r//128 % ..., partition p=r%128. All DMAs are fully
contiguous DRAM reads.

Per-image sums (for topk) = per-partition ACT Square accum_out columns, one
col per col-block; host folds groups of 16 rows into images.
"""

import functools
import sys

import numpy as np

for _p in ("/opt/trn_rl_repo",):
    if _p not in sys.path:
        sys.path.insert(0, _p)

import concourse.bass as bass
import concourse.bacc as bacc
import concourse.mybir as mybir
from concourse import tile
from concourse.bass_utils import run_bass_kernel_spmd

f32 = mybir.dt.float32
bf16 = mybir.dt.bfloat16
OP = mybir.AluOpType
AF = mybir.ActivationFunctionType

B, K, H, W = 64, 17, 96, 72
HW = H * W                # 6912
NCORES = 8
BL = B // NCORES          # 8 samples per core
P = 128
E = 432                   # contiguous elements per row
RPI = HW // E             # 16 rows per image
NIMG = BL * K             # 136 images per core
NROW = NIMG * RPI         # 2176 rows = 17 col-blocks of 128
NCB = NROW // P           # 17 col-blocks
NTOT = NROW * E           # 940032 elements per volume per core

COMPUTE_DT = bf16
NSUP = 3 if COMPUTE_DT == bf16 else 6   # super-blocks (SBUF capacity)

# out_sb column layout
S3_0, S4X_0, S4Y_0 = 0, NCB, 2 * NCB           # per col-block cols (0..51)
A1_0 = 3 * NCB                                  # per super-block cols
A2X_0 = A1_0 + NSUP
A2Y_0 = A2X_0 + NSUP
NCOL = A2Y_0 + NSUP

VOLS = ("o11", "o1x", "o1y", "o21", "o2x", "o2y", "t1", "t2x", "t2y")


def _sup_blocks():
    """Split NCB col-blocks into NSUP contiguous groups."""
    base, rem = divmod(NCB, NSUP)
    out, cb0 = [], 0
    for s in range(NSUP):
        nb = base + (1 if s < rem else 0)
        out.append((cb0, nb))
        cb0 += nb
    return out


def build_nc():
    nc = bacc.Bacc(
        "TRN2",
        target_bir_lowering=False,
        debug=False,
        num_devices=NCORES,
    )
    dram = {v: nc.dram_tensor(v, [NTOT], f32, kind="ExternalInput").ap() for v in VOLS}
    outd = nc.dram_tensor("out", [P, NCOL], f32, kind="ExternalOutput").ap()

    cdt = COMPUTE_DT
    sup = _sup_blocks()
    fd_max = max(nb for _, nb in sup) * E

    with tile.TileContext(nc) as tc:
        with (
            tc.tile_pool(name="io", bufs=2) as io,
            tc.tile_pool(name="sc", bufs=2) as sc,
            tc.tile_pool(name="res", bufs=1) as resp,
        ):
            out_sb = resp.tile([P, NCOL], f32)

            for s, (cb0, nb) in enumerate(sup):
                fd = nb * E
                bt = {}
                for v in VOLS:
                    tl = io.tile([P, fd_max], cdt, tag=v)
                    # host pre-permutes each super-block so partition p's data
                    # is one contiguous fd-element DRAM run (big descriptors)
                    src = dram[v][cb0 * P * E : (cb0 + nb) * P * E].rearrange(
                        "(p f) -> p f", f=fd
                    )
                    dst = tl[:, :fd]
                    if cdt == f32:
                        nc.sync.dma_start(dst, src)
                    else:
                        nc.gpsimd.dma_start(dst, src)  # casting DMA f32->bf16
                    bt[v] = tl

                def tt(tag_or_tile, a, b, op):
                    if isinstance(tag_or_tile, str):
                        t = sc.tile([P, fd_max], cdt, tag=tag_or_tile)
                    else:
                        t = tag_or_tile  # in-place: overwrite an existing tile
                    nc.vector.tensor_tensor(t[:, :fd], a[:, :fd], b[:, :fd], op=op)
                    return t

                def sq_accum_global(t, col):
                    """ACT square (in-place), accum -> one col."""
                    nc.scalar.activation(
                        t[:, :fd], t[:, :fd], AF.Square,
                        accum_out=out_sb[:, col : col + 1],
                    )

                def sq_accum_blocks(t, col0):
                    """ACT square per col-block (in-place), accum -> col per block."""
                    for b in range(nb):
                        sl = slice(b * E, (b + 1) * E)
                        nc.scalar.activation(
                            t[:, sl], t[:, sl], AF.Square,
                            accum_out=out_sb[:, col0 + cb0 + b : col0 + cb0 + b + 1],
                        )

                # S1: sum (o11 - t1)^2
                d1 = tt("wA", bt["o11"], bt["t1"], OP.subtract)
                sq_accum_global(d1, A1_0 + s)

                # S2x/S2y: sum (o1 * t1 - t2)^2
                for o_nm, t_nm, col in (("o1x", "t2x", A2X_0 + s), ("o1y", "t2y", A2Y_0 + s)):
                    ee = tt("wB" + o_nm, bt[o_nm], bt["t1"], OP.mult)
                    ee = tt(ee, ee, bt[t_nm], OP.subtract)
                    sq_accum_global(ee, col)

                # S3: per-image sum (o21 - t1)^2
                d3 = tt("wD", bt["o21"], bt["t1"], OP.subtract)
                sq_accum_blocks(d3, S3_0)

                # S4x/S4y: per-image sum (t1 * (o2 - t2))^2 -- one big ACT
                # square + one DVE segmented reduce per chain (keeps ACT off
                # the critical tail; S3 stays on ACT small squares)
                for o_nm, t_nm, col0 in (("o2x", "t2x", S4X_0), ("o2y", "t2y", S4Y_0)):
                    gg = tt("wE" + o_nm, bt[o_nm], bt[t_nm], OP.subtract)
                    gg = tt(gg, gg, bt["t1"], OP.mult)
                    nc.scalar.activation(gg[:, :fd], gg[:, :fd], AF.Square)
                    nc.vector.tensor_reduce(
                        out_sb[:, col0 + cb0 : col0 + cb0 + nb],
                        gg[:, :fd].rearrange("p (cb e) -> p cb e", e=E),
                        axis=mybir.AxisListType.X,
                        op=OP.add,
                    )

            nc.sync.dma_start(outd[:, :], out_sb[:, :])

    nc.compile()
    return nc


@functools.lru_cache(maxsize=1)
def _built():
    return build_nc()


def make_in_maps(out1_1, out1_2, out2_1, out2_2, targets1, targets2):
    sup = _sup_blocks()

    def flat(a):
        """Flatten + permute per super-block: [nb*128, E] -> [128, nb*E] so
        each partition's tile data is one contiguous DRAM run."""
        rows = np.ascontiguousarray(a, np.float32).reshape(NROW, E)
        parts = [
            rows[cb0 * P : (cb0 + nb) * P]
            .reshape(nb, P, E).transpose(1, 0, 2).reshape(-1)
            for cb0, nb in sup
        ]
        return np.concatenate(parts)

    in_maps = []
    for c in range(NCORES):
        sl = slice(c * BL, (c + 1) * BL)
        in_maps.append({
            "o11": flat(out1_1[sl]),
            "o1x": flat(out1_2[sl, :K]),
            "o1y": flat(out1_2[sl, K:]),
            "o21": flat(out2_1[sl]),
            "o2x": flat(out2_2[sl, :K]),
            "o2y": flat(out2_2[sl, K:]),
            "t1": flat(targets1[sl]),
            "t2x": flat(targets2[sl, :K]),
            "t2y": flat(targets2[sl, K:]),
        })
    return in_maps


def combine(core_outs, weights):
    """core_outs: list of NCORES arrays [P, NCOL] f32 -> scalar loss (f64)."""
    N = B * K * HW
    S1 = S2x = S2y = 0.0
    S3 = np.zeros((B, K))
    S4 = np.zeros((B, K))

    def per_image(o, col0):
        # col b, partition p -> row r = b*128+p -> image i = r//16
        flat_rows = o[:, col0 : col0 + NCB].T.reshape(NROW)
        return flat_rows.reshape(NIMG, RPI).sum(axis=1).reshape(BL, K)

    for c, o in enumerate(core_outs):
        o = o.astype(np.float64)
        S1 += o[:, A1_0 : A1_0 + NSUP].sum()
        S2x += o[:, A2X_0 : A2X_0 + NSUP].sum()
        S2y += o[:, A2Y_0 : A2Y_0 + NSUP].sum()
        csl = slice(c * BL, (c + 1) * BL)
        S3[csl] = per_image(o, S3_0)
        S4[csl] = per_image(o, S4X_0) + per_image(o, S4Y_0)

    l11 = S1 / N
    l12 = (S2x + S2y) / N
    pk = S3 / (2.0 * B)
    idx = np.argsort(-pk, axis=1, kind="stable")[:, : K // 2]
    mask = np.zeros((B, K))
    np.put_along_axis(mask, idx, 1.0, axis=1)
    l21 = (pk * mask).sum() / (B * K)
    l22 = (S4 * mask).sum() / N
    w = np.asarray(weights, np.float64)
    return (l11 + l21) * w[0] + (l12 + 5.0 * l22) * w[1]


def run_device(in_maps, trace=False, **kw):
    nc = _built()
    return run_bass_kernel_spmd(nc, in_maps, core_ids=list(range(NCORES)),
                                trace=trace, **kw)


def kernel(out1_1, out1_2, out2_1, out2_2, targets1, targets2, weights):
    in_maps = make_in_maps(out1_1, out1_2, out2_1, out2_2, targets1, targets2)
    res = run_device(in_maps)
    core_outs = [res.results[i]["out"] for i in range(NCORES)]
    loss = combine(core_outs, np.asarray(weights, np.float32))
    return np.float32(loss)


# revision 14
# speedup vs baseline: 1.0476x; 1.0476x over previous
"""Trainium2 Bass kernel for nn_HRPLoss (topk_masking).

Contract: kernel(**inputs) takes FULL unsharded numpy inputs and returns the
FULL output (scalar f32 loss). Pure data parallel: batch dim sharded over 8
NeuronCores; device computes all the heavy streaming reductions; host does the
tiny per-sample top-k masking + final scalar combine (64x17 values).

Layout: each per-core logical volume (9 of them: out1_1, out1_2{x,y}, out2_1,
out2_2{x,y}, targets1, targets2{x,y}) is a flat [BL*K*HW] = [940032] f32 array
viewed as [2176, 432] rows (row r = image i=r//16, chunk ch=r%16; one image =
16 rows of 432 contiguous elements). Tiles are [128, nb*432] bf16 (cast in the
DMA), row r = col-block cb=r//128 % ..., partition p=r%128. All DMAs are fully
contiguous DRAM reads.

Per-image sums (for topk) = per-partition ACT Square accum_out columns, one
col per col-block; host folds groups of 16 rows into images.
"""

import functools
import sys

import numpy as np

for _p in ("/opt/trn_rl_repo",):
    if _p not in sys.path:
        sys.path.insert(0, _p)

import concourse.bass as bass
import concourse.bacc as bacc
import concourse.mybir as mybir
from concourse import tile
from concourse.bass_utils import run_bass_kernel_spmd

f32 = mybir.dt.float32
bf16 = mybir.dt.bfloat16
OP = mybir.AluOpType
AF = mybir.ActivationFunctionType

B, K, H, W = 64, 17, 96, 72
HW = H * W                # 6912
NCORES = 8
BL = B // NCORES          # 8 samples per core
P = 128
E = 432                   # contiguous elements per row
RPI = HW // E             # 16 rows per image
NIMG = BL * K             # 136 images per core
NROW = NIMG * RPI         # 2176 rows = 17 col-blocks of 128
NCB = NROW // P           # 17 col-blocks
NTOT = NROW * E           # 940032 elements per volume per core

COMPUTE_DT = bf16
NSUP = 3 if COMPUTE_DT == bf16 else 6   # super-blocks (SBUF capacity)

# out_sb column layout
S3_0, S4X_0, S4Y_0 = 0, NCB, 2 * NCB           # per col-block cols (0..51)
A1_0 = 3 * NCB                                  # per super-block cols
A2X_0 = A1_0 + NSUP
A2Y_0 = A2X_0 + NSUP
NCOL = A2Y_0 + NSUP

VOLS = ("o11", "o1x", "o1y", "o21", "o2x", "o2y", "t1", "t2x", "t2y")


def _sup_blocks():
    """Split NCB col-blocks into NSUP contiguous groups."""
    base, rem = divmod(NCB, NSUP)
    out, cb0 = [], 0
    for s in range(NSUP):
        nb = base + (1 if s < rem else 0)
        out.append((cb0, nb))
        cb0 += nb
    return out


def build_nc():
    nc = bacc.Bacc(
        "TRN2",
        target_bir_lowering=False,
        debug=False,
        num_devices=NCORES,
    )
    dram = {v: nc.dram_tensor(v, [NTOT], f32, kind="ExternalInput").ap() for v in VOLS}
    outd = nc.dram_tensor("out", [P, NCOL], f32, kind="ExternalOutput").ap()

    cdt = COMPUTE_DT
    sup = _sup_blocks()
    fd_max = max(nb for _, nb in sup) * E

    with tile.TileContext(nc) as tc:
        with (
            tc.tile_pool(name="io", bufs=2) as io,
            tc.tile_pool(name="sc", bufs=2) as sc,
            tc.tile_pool(name="res", bufs=1) as resp,
        ):
            out_sb = resp.tile([P, NCOL], f32)

            for s, (cb0, nb) in enumerate(sup):
                fd = nb * E
                bt = {}
                for v in VOLS:
                    tl = io.tile([P, fd_max], cdt, tag=v)
                    # host pre-permutes each super-block so partition p's data
                    # is one contiguous fd-element DRAM run (big descriptors)
                    src = dram[v][cb0 * P * E : (cb0 + nb) * P * E].rearrange(
                        "(p f) -> p f", f=fd
                    )
                    dst = tl[:, :fd]
                    if cdt == f32:
                        nc.sync.dma_start(dst, src)
                    else:
                        nc.gpsimd.dma_start(dst, src)  # casting DMA f32->bf16
                    bt[v] = tl

                def tt(tag_or_tile, a, b, op):
                    if isinstance(tag_or_tile, str):
                        t = sc.tile([P, fd_max], cdt, tag=tag_or_tile)
                    else:
                        t = tag_or_tile  # in-place: overwrite an existing tile
                    nc.vector.tensor_tensor(t[:, :fd], a[:, :fd], b[:, :fd], op=op)
                    return t

                def sq_accum_global(t, col):
                    """ACT square (in-place), accum -> one col."""
                    nc.scalar.activation(
                        t[:, :fd], t[:, :fd], AF.Square,
                        accum_out=out_sb[:, col : col + 1],
                    )

                def sq_accum_blocks(t, col0):
                    """ACT square per col-block (in-place), accum -> col per block."""
                    for b in range(nb):
                        sl = slice(b * E, (b + 1) * E)
                        nc.scalar.activation(
                            t[:, sl], t[:, sl], AF.Square,
                            accum_out=out_sb[:, col0 + cb0 + b : col0 + cb0 + b + 1],
                        )

                # S1: sum (o11 - t1)^2
                d1 = tt("wA", bt["o11"], bt["t1"], OP.subtract)
                sq_accum_global(d1, A1_0 + s)

                # S2x/S2y: sum (o1 * t1 - t2)^2
                for o_nm, t_nm, col in (("o1x", "t2x", A2X_0 + s), ("o1y", "t2y", A2Y_0 + s)):
                    ee = tt("wB" + o_nm, bt[o_nm], bt["t1"], OP.mult)
                    ee = tt(ee, ee, bt[t_nm], OP.subtract)
                    sq_accum_global(ee, col)

                # S3: per-image sum (o21 - t1)^2
                d3 = tt("wD", bt["o21"], bt["t1"], OP.subtract)
                sq_accum_blocks(d3, S3_0)

                # S4x/S4y: per-image sum (t1 * (o2 - t2))^2
                for o_nm, t_nm, col0 in (("o2x", "t2x", S4X_0), ("o2y", "t2y", S4Y_0)):
                    gg = tt("wE" + o_nm, bt[o_nm], bt[t_nm], OP.subtract)
                    gg = tt(gg, gg, bt["t1"], OP.mult)
                    sq_accum_blocks(gg, col0)

            nc.sync.dma_start(outd[:, :], out_sb[:, :])

    nc.compile()
    return nc


@functools.lru_cache(maxsize=1)
def _built():
    return build_nc()


def make_in_maps(out1_1, out1_2, out2_1, out2_2, targets1, targets2):
    sup = _sup_blocks()

    def flat(a):
        """Flatten + permute per super-block: [nb*128, E] -> [128, nb*E] so
        each partition's tile data is one contiguous DRAM run."""
        rows = np.ascontiguousarray(a, np.float32).reshape(NROW, E)
        parts = [
            rows[cb0 * P : (cb0 + nb) * P]
            .reshape(nb, P, E).transpose(1, 0, 2).reshape(-1)
            for cb0, nb in sup
        ]
        return np.concatenate(parts)

    in_maps = []
    for c in range(NCORES):
        sl = slice(c * BL, (c + 1) * BL)
        in_maps.append({
            "o11": flat(out1_1[sl]),
            "o1x": flat(out1_2[sl, :K]),
            "o1y": flat(out1_2[sl, K:]),
            "o21": flat(out2_1[sl]),
            "o2x": flat(out2_2[sl, :K]),
            "o2y": flat(out2_2[sl, K:]),
            "t1": flat(targets1[sl]),
            "t2x": flat(targets2[sl, :K]),
            "t2y": flat(targets2[sl, K:]),
        })
    return in_maps


def combine(core_outs, weights):
    """core_outs: list of NCORES arrays [P, NCOL] f32 -> scalar loss (f64)."""
    N = B * K * HW
    S1 = S2x = S2y = 0.0
    S3 = np.zeros((B, K))
    S4 = np.zeros((B, K))

    def per_image(o, col0):
        # col b, partition p -> row r = b*128+p -> image i = r//16
        flat_rows = o[:, col0 : col0 + NCB].T.reshape(NROW)
        return flat_rows.reshape(NIMG, RPI).sum(axis=1).reshape(BL, K)

    for c, o in enumerate(core_outs):
        o = o.astype(np.float64)
        S1 += o[:, A1_0 : A1_0 + NSUP].sum()
        S2x += o[:, A2X_0 : A2X_0 + NSUP].sum()
        S2y += o[:, A2Y_0 : A2Y_0 + NSUP].sum()
        csl = slice(c * BL, (c + 1) * BL)
        S3[csl] = per_image(o, S3_0)
        S4[csl] = per_image(o, S4X_0) + per_image(o, S4Y_0)

    l11 = S1 / N
    l12 = (S2x + S2y) / N
    pk = S3 / (2.0 * B)
    idx = np.argsort(-pk, axis=1, kind="stable")[:, : K // 2]
    mask = np.zeros((B, K))
    np.put_along_axis(mask, idx, 1.0, axis=1)
    l21 = (pk * mask).sum() / (B * K)
    l22 = (S4 * mask).sum() / N
    w = np.asarray(weights, np.float64)
    return (l11 + l21) * w[0] + (l12 + 5.0 * l22) * w[1]


def run_device(in_maps, trace=False, **kw):
    nc = _built()
    return run_bass_kernel_spmd(nc, in_maps, core_ids=list(range(NCORES)),
                                trace=trace, **kw)


def kernel(out1_1, out1_2, out2_1, out2_2, targets1, targets2, weights):
    in_maps = make_in_maps(out1_1, out1_2, out2_1, out2_2, targets1, targets2)
    res = run_device(in_maps)
    core_outs = [res.results[i]["out"] for i in range(NCORES)]
    loss = combine(core_outs, np.asarray(weights, np.float32))
    return np.float32(loss)
